# revision 16
# baseline (speedup 1.0000x reference)
"""CRF loss (nn_CRF_52664888984293) on 8 Trainium2 NeuronCores.

Strategy: data-parallel over batch B=1024 -> 128 per core. The partition
function (forward algorithm) is computed on device in the exp domain:
    alpha_{t+1} = exp(h_t - R0) * (W @ alpha_t),   W = exp(trans)
with state alpha kept as a [C=64 partitions, B_local=128 free] bf16 tile.
Each step is one PE matmul (augmented with an extra "sigma" output row
sum_j exp(trans[EOS,j]) * alpha[j,b]) plus one DVE elementwise multiply.
sigma is snapshotted every step; the host reconstructs
    Z_s[b] = log sigma_s[b] + s*R0 + (renorm corrections)
and picks s = length[b].  A per-batch renorm (divide alpha by a stale
sigma snapshot) every 32 steps keeps values in fp32/bf16 range.

The gold-path score (pure gather over h plus tiny trans lookups) is
computed on the host; the device still reads all of h, so the kernel's
memory roofline is unchanged.
"""

import threading
from contextlib import ExitStack

import ml_dtypes
import numpy as np

import concourse.bass as bass
import concourse.bacc as bacc
import concourse.tile as tile
from concourse import mybir
from concourse.bass_utils import run_bass_kernel_spmd

T, B, C = 512, 1024, 64
PAD_IDX, SOS_IDX, EOS_IDX = 0, 1, 2
NCORES = 8
BL = B // NCORES          # 128 batch elements per core
CH = 16                   # time steps per DMA/exp chunk
NCH = T // CH             # 32 chunks
R0 = 4.6                  # constant per-step log-shift baked into exp()
RENORM_MS = tuple(range(32, 512, 32))  # steps (matmul idx m) with renorm
RENORM_LAG = 11           # renorm at m divides by sigma snapshot slot m-11

_cache = {}
_cache_lock = threading.Lock()
last_results = None       # BassKernelResults of the most recent run (for test harness)


def _build_program():
    f32 = mybir.dt.float32
    bf16 = mybir.dt.bfloat16
    nc = bacc.Bacc("TRN2", target_bir_lowering=False, debug=False, num_devices=NCORES)

    hc = nc.dram_tensor("hc", [NCH, C, CH * BL], f32, kind="ExternalInput").ap()
    wsig_d = nc.dram_tensor("wsig", [C, C + 1], bf16, kind="ExternalInput").ap()
    a0_d = nc.dram_tensor("a0", [C, BL], bf16, kind="ExternalInput").ap()
    sig_d = nc.dram_tensor("sig", [128, 512], f32, kind="ExternalOutput").ap()

    with ExitStack() as ctx:
        tc = ctx.enter_context(tile.TileContext(nc))
        consts = ctx.enter_context(tc.tile_pool(name="consts", bufs=1))
        hpool = ctx.enter_context(tc.tile_pool(name="hch", bufs=3))
        gpool = ctx.enter_context(tc.tile_pool(name="gch", bufs=3))
        apool = ctx.enter_context(tc.tile_pool(name="alpha", bufs=3))
        rpool = ctx.enter_context(tc.tile_pool(name="renorm", bufs=2))
        srows = ctx.enter_context(tc.tile_pool(name="sigrow", bufs=4))
        psum = ctx.enter_context(tc.tile_pool(name="mm", bufs=5, space="PSUM"))
        bcps = ctx.enter_context(tc.tile_pool(name="bc", bufs=2, space="PSUM"))

        wsig_sb = consts.tile([C, C + 1], bf16)
        nc.sync.dma_start(out=wsig_sb[:], in_=wsig_d)
        ones_sb = consts.tile([1, C], f32)
        nc.vector.memset(ones_sb[:], 1.0)
        nbias_sb = consts.tile([C, 1], f32)
        nc.vector.memset(nbias_sb[:], -R0)

        alpha = apool.tile([C, BL], bf16, tag="alpha")
        nc.sync.dma_start(out=alpha[:], in_=a0_d)

        g_tiles = [None] * NCH

        def load_chunk(p):
            h_t = hpool.tile([C, CH * BL], f32, tag="hch")
            nc.sync.dma_start(out=h_t[:], in_=hc[p])
            g_t = gpool.tile([C, CH * BL], bf16, tag="gch")
            nc.scalar.activation(
                out=g_t[:], in_=h_t[:],
                func=mybir.ActivationFunctionType.Exp, bias=nbias_sb[:],
            )
            g_tiles[p] = g_t

        load_chunk(0)

        cur_psum = None
        psum_tiles = {}  # group -> tile (for renorm reads of older sigma rows)
        for m in range(1, T + 2):  # matmuls 1..513
            slot = m - 2
            if m == 1 or (slot % 4) == 0:
                cur_psum = psum.tile([C + 1, 512], f32, tag="mm")
                psum_tiles[slot // 4] = cur_psum
            col = 0 if m == 1 else slot % 4
            nc.tensor.matmul(
                cur_psum[:, col * BL:(col + 1) * BL],
                lhsT=wsig_sb[:],
                rhs=alpha[:],
                start=True, stop=True,
            )
            if m <= T:
                j = m - 1
                p, k = j // CH, j % CH
                if k == 0 and p + 1 < NCH and g_tiles[p + 1] is None:
                    load_chunk(p + 1)
                g_slice = g_tiles[p][:, k * BL:(k + 1) * BL]
                if m in RENORM_MS:
                    q = m - RENORM_LAG
                    srow = psum_tiles[q // 4][C:C + 1, (q % 4) * BL:(q % 4 + 1) * BL]
                    rcp = rpool.tile([1, BL], f32, tag="rcp")
                    nc.vector.reciprocal(out=rcp[:], in_=srow)
                    bc = bcps.tile([C, BL], f32, tag="bc")
                    nc.tensor.matmul(bc[:], lhsT=ones_sb[:], rhs=rcp[:],
                                     start=True, stop=True)
                    gn = rpool.tile([C, BL], bf16, tag="gn")
                    nc.vector.tensor_mul(gn[:], g_slice, bc[:])
                    g_slice = gn[:]
                alpha = apool.tile([C, BL], bf16, tag="alpha")
                nc.vector.tensor_mul(alpha[:], cur_psum[:C, col * BL:(col + 1) * BL], g_slice)
            if slot >= 0 and (slot % 4 == 3 or m == T + 1):
                g = slot // 4
                srow_sb = srows.tile([1, 512], f32, tag="sigrow")
                nc.scalar.activation(
                    out=srow_sb[:], in_=cur_psum[C:C + 1, :],
                    func=mybir.ActivationFunctionType.Copy,
                )
                nc.sync.dma_start(out=sig_d[g:g + 1, :], in_=srow_sb[:])

    nc.compile()
    return nc


def _get_program():
    with _cache_lock:
        if "nc" not in _cache:
            _cache["nc"] = _build_program()
        return _cache["nc"]


def kernel(h, y0, mask, trans):
    global last_results
    h = np.ascontiguousarray(np.asarray(h, dtype=np.float32))
    y0 = np.asarray(y0).astype(np.int64)
    mask = np.asarray(mask, dtype=np.float32)
    trans = np.asarray(trans, dtype=np.float32)

    lengths = mask.sum(0).astype(np.int64)            # [B], in [256, 512]
    W = np.exp(trans.astype(np.float64))
    wsig = np.concatenate([W.T, W[EOS_IDX][:, None]], axis=1)  # [C, C+1]
    wsig_bf = wsig.astype(ml_dtypes.bfloat16)
    a0 = np.zeros((C, BL), dtype=ml_dtypes.bfloat16)
    a0[SOS_IDX] = 1.0

    in_maps = []
    for core in range(NCORES):
        sl = slice(core * BL, (core + 1) * BL)
        # [T, BL, C] -> [NCH, C, CH, BL] -> [NCH, C, CH*BL]
        hcore = h[:, sl, :].reshape(NCH, CH, BL, C).transpose(0, 3, 1, 2)
        hcore = np.ascontiguousarray(hcore).reshape(NCH, C, CH * BL)
        in_maps.append({"hc": hcore, "wsig": wsig_bf, "a0": a0})

    nc = _get_program()
    res = run_bass_kernel_spmd(nc, in_maps, list(range(NCORES)))
    last_results = res

    # ---- host: reconstruct log-partition per batch element ----
    z = np.zeros(B, dtype=np.float64)
    for core in range(NCORES):
        sig = np.asarray(res.results[core]["sig"], dtype=np.float64)  # [128, 512]
        logsig = np.log(sig.reshape(128, 4, BL)).transpose(0, 1, 2).reshape(512, BL)
        # slot s-1 (s = steps done) lives at [slot//4, (slot%4)*BL + b]
        cvec = np.zeros(BL, dtype=np.float64)
        zz = np.empty((T, BL), dtype=np.float64)
        for m in range(2, T + 2):
            if (m - 1) in RENORM_MS:
                q = (m - 1) - RENORM_LAG
                cvec = cvec + logsig[q]
            s = m - 1
            zz[s - 1] = logsig[m - 2] + s * R0 + cvec
        sl = slice(core * BL, (core + 1) * BL)
        z[sl] = zz[lengths[sl] - 1, np.arange(BL)]

    # ---- host: gold-path score (tiny gather; device already reads all of h) ----
    yc, yp = y0[1:T], y0[:T - 1]
    emit = np.take_along_axis(h[:T - 1], yc[:, :, None], axis=2)[..., 0]
    tr = trans[yc, yp]
    S = ((emit.astype(np.float64) + tr) * mask[:T - 1]).sum(0)
    S = S + trans[PAD_IDX, y0[lengths, np.arange(B)]]

    loss = np.mean(z - S)
    return np.array(loss, dtype=np.float32)
